# revision 47
# baseline (speedup 1.0000x reference)
# Bloom parallel attention block on 8 trn2 NeuronCores, tensor-parallel over
# heads (2 heads per core).  Feature-major layouts throughout.
#
# v2: fp8 DoubleRow on every contraction>=256 matmul (QKV, ctx, denominator,
# dense) — two 128-deep k-tiles per PE instruction.  Scores stay bf16 (softmax
# is the accuracy-critical path).  Weights are prescaled x32 on host so their
# ~N(0, 1/2048) entries clear e4m3's 2^-6 min-normal; the 1/32 is folded into
# the PSUM evacuation (DVE tensor_scalar mult+add).  Probs are e5m2 (range:
# exp(scores) spans e4m3's +-240 but not e5m2's +-57344).  The ctx AllGather
# moves fp8 bytes (half of bf16), split into 8 per-(batch,q-chunk) ops so
# each triggers as soon as its two head-blocks finish and the serial CC
# stream stays spread out (the last gather is what exposes the tail).
#
# Scheduling notes (hard-won):
#  - DMAs that WAIT on a collective's semaphore must ride the gpsimd queue.
#    The static scheduler places them optimistically early in whatever engine
#    stream they're on; on the strict-FIFO scalar/sync queues they then
#    head-of-line block Exp / ctx-write instructions for the gather's full
#    latency (a 20-37us PE stall).  On gpsimd everything behind them (later
#    collective triggers) is already dependency-ordered after the gather.
#  - hid/wq/wd are host-pre-swizzled to per-partition-contiguous layouts:
#    scattered 512-768B DMA segments starved the phase-1 weave.
#  - wd/rs load at phase-2 start: phase 1 is HBM-tight, and mid-phase-2
#    collides with the first gather's fabric traffic.
#
# Per core r (heads 2r, 2r+1):
#   QKV matmul -> Q^T/K^T [d, s] (bf16, inv_norm applied at evacuation) and
#   V^T [d, s] per batch in SBUF.
#   V^T is transposed on the PE to V [s, d] and scaled by exp(alibi[k]) on
#   evacuation; the softmax-denominator matmul weights are exp(alibi[k])
#   broadcast columns ("ones'").  This folds alibi in MULTIPLICATIVELY:
#     exp(s + a) * mask = exp(s) * mask * exp(a)
#   so the ACT Exp needs no per-k-tile bias and can process two k-tiles per
#   instruction.
#   attention (per b, head hl, 512-wide q-chunk qc), scores transposed [k, q]:
#     scores^T = K^T_tile.T @ Q^T       per k-tile  (PE bf16, fp32 psum)
#     exp(scores) per k-tile            (ACT, bf16 out; per-512 psum halves
#                                        so scores(kp+1) never stalls on ACT)
#     * mask01^T                        (DVE, 0/1 mask -> e5m2 probs; tried
#                                        additive pre-exp mask: DVE lands in
#                                        the sco->exp critical path, -85us)
#     ctx^T += V'_pair.T @ probs        (PE fp8 DoubleRow, 8 k-pair steps)
#     sum   += ones'_pair.T @ probs     (PE fp8 DoubleRow, denominator)
#     ctx^T *= 1/sum -> e4m3 -> DMA to cc chunk
#   Pipelining: QKV(b1) matmuls are interleaved into attention(b0) k-loops,
#   dense matmuls into attention(b1) k-loops, so the PE never idles.  ctx
#   is AllGathered (fp8) in 8 per-(batch,q-chunk) ops; dense consumes each
#   as it lands (only the last chunk's dense remains after the attention).
#   dense: out^T[o_local, s] = wdT_pair.T @ ctx^T_full (DoubleRow), then
#     out = psum/32 + (residual^T + b_dense)  (DVE scalar_tensor_tensor)
import os
import sys

import numpy as np

if "/opt/trn_rl_repo" not in sys.path:
    sys.path.insert(0, "/opt/trn_rl_repo")

import ml_dtypes

import concourse.bass as bass
import concourse.mybir as mybir
import concourse.tile as tile
from concourse import bacc, bass_utils

B, S, H, NH = 2, 2048, 2048, 16
HD = H // NH            # 128
NCORES = 8
HPC = NH // NCORES      # heads per core = 2
OSH = 3 * H // NCORES   # qkv output rows per core = 768
DSH = H // NCORES       # dense output cols per core = 256
P = 128
F32 = mybir.dt.float32
BF16 = mybir.dt.bfloat16
F8 = mybir.dt.float8e4
F8P = mybir.dt.float8e5
DRM = mybir.MatmulPerfMode.DoubleRow
AF = mybir.ActivationFunctionType
ALU = mybir.AluOpType
NPBF16 = ml_dtypes.bfloat16
NPE4 = ml_dtypes.float8_e4m3
WS = 32.0               # host weight prescale (fp8 subnormal avoidance)
INV = 1.0 / np.sqrt(HD)


def build_nc():
    nc = bacc.Bacc(
        "TRN2",
        target_bir_lowering=False,
        debug=False,
        num_devices=NCORES,
    )

    # hidden states pre-swizzled on host to [(b sc p), (hp a q)] so each
    # 512-wide s-chunk is one fully-contiguous 1MB DMA
    hidT = nc.dram_tensor("hidT", [B * 4 * P, 8 * 2 * 512], F8, kind="ExternalInput").ap()
    # weights pre-swizzled on host to the SBUF tile layout [p, ht, o] so the
    # load is one fully-contiguous DMA per partition
    wqkvT = nc.dram_tensor("wqkvT", [P, 16 * OSH], F8, kind="ExternalInput").ap()
    bqkv = nc.dram_tensor("bqkv", [P, 6], F32, kind="ExternalInput").ap()
    mask01T = nc.dram_tensor("mask01T", [S, S], F8, kind="ExternalInput").ap()
    alibi_e = nc.dram_tensor("alibi_e", [P, 2 * HPC * 16], F32, kind="ExternalInput").ap()
    wdT = nc.dram_tensor("wdT", [P, 16 * DSH], F8, kind="ExternalInput").ap()
    residT = nc.dram_tensor("residT", [DSH, B * S], F32, kind="ExternalInput").ap()
    # host-prebuilt denominator weights exp(alibi[k]) broadcast to 128 cols
    owe = nc.dram_tensor("owe", [P, B * HPC * 16 * P], F8, kind="ExternalInput").ap()
    eye = nc.dram_tensor("eye", [P, P], BF16, kind="ExternalInput").ap()
    outT = nc.dram_tensor("outT", [DSH, B * S], F32, kind="ExternalOutput").ap()

    with tile.TileContext(nc) as tc:
        ccg = [list(range(NCORES))]
        with (
            tc.tile_pool(name="const", bufs=1) as constp,
            tc.tile_pool(name="dram", bufs=1, space="DRAM") as dramp,
        ):
            # consts ride the idle scalar queue so the gpsimd queue head is
            # free for the wq bulk loads (first-matmul critical path)
            bq_sb = constp.tile([P, 6], F32)
            nc.scalar.dma_start(bq_sb, bqkv)
            ale_sb = constp.tile([P, 2 * HPC * 16], F32)
            nc.scalar.dma_start(ale_sb, alibi_e)
            eye_sb = constp.tile([P, P], BF16)
            nc.scalar.dma_start(eye_sb, eye)

            # ctx gather chunks: one 512-col gather per (batch, q-chunk) —
            # small ops trigger early and keep the serial CC stream spread
            # out, so the phase-3 gather chain never queues behind a big op
            # and the tail drains quickly.
            cc_spec = [(4, S // 4), (4, S // 4)]
            cc_in = [
                [
                    dramp.tile([HPC * HD, w], F8, name=f"cc_in{b}{i}")
                    for i in range(n)
                ]
                for b, (n, w) in enumerate(cc_spec)
            ]
            cc_out = [
                [
                    dramp.tile([H, w], F8, addr_space="Shared", name=f"cc_out{b}{i}")
                    for i in range(n)
                ]
                for b, (n, w) in enumerate(cc_spec)
            ]

            def dma_ctx(b, qc, hl, ctxn_t):
                n, w = cc_spec[b]
                chunk, qq = divmod(qc, 4 // n)
                nc.sync.dma_start(
                    cc_in[b][chunk][hl * P : (hl + 1) * P, qq * 512 : (qq + 1) * 512],
                    ctxn_t,
                )

            def all_gather(b, chunk):
                nc.gpsimd.collective_compute(
                    "AllGather", mybir.AluOpType.bypass, replica_groups=ccg,
                    ins=[cc_in[b][chunk].opt()], outs=[cc_out[b][chunk].opt()],
                )

            with (
                tc.tile_pool(name="mask", bufs=1) as maskp,
                tc.tile_pool(name="qk1", bufs=1) as qk1p,
                tc.tile_pool(name="vt", bufs=1) as vtp,
                tc.tile_pool(name="v1", bufs=1) as v1p,
                tc.tile_pool(name="ow0", bufs=1) as ow0p,
                tc.tile_pool(name="ow1", bufs=1) as ow1p,
                tc.tile_pool(name="dw", bufs=1) as dwp,
                tc.tile_pool(name="dctx", bufs=12) as dctxp,
            ):
                mask_sb = maskp.tile([P, 16, S], F8)
                qk_sbs = [None, qk1p.tile([P, 2 * HPC, S], BF16, name="qksb1")]
                v_sbs = [None, v1p.tile([P, HPC, 16, P], F8, name="vsb1")]
                ow_sbs = [
                    ow0p.tile([P, HPC, 16, P], F8, name="owsb0"),
                    ow1p.tile([P, HPC, 16, P], F8, name="owsb1"),
                ]
                wd_sb = dwp.tile([P, 16, DSH], F8)
                rs_sb = dwp.tile([P, 2, B * S], F32)

                def dense_src(sc):
                    """cc_out chunk + column offset for output chunk sc."""
                    if sc < 4:
                        return cc_out[0][sc], 0
                    return cc_out[1][sc - 4], 0

                def dense_load(sc):
                    """Stage the gathered ctx for output chunk sc into SBUF:
                    4 tiles of 4 h-tiles each.  MUST ride the gpsimd queue:
                    these DMAs wait on a collective's completion semaphore,
                    and the static scheduler places them optimistically early
                    in whatever engine stream they're on — on scalar/sync the
                    strict-FIFO queue then head-of-line blocks Exp/ctx-write
                    instructions for the gather's full latency.  On gpsimd
                    everything behind them (later collective triggers) is
                    already dependency-ordered after the gather they wait on."""
                    src, col_off = dense_src(sc)
                    tiles = []
                    for i in range(4):
                        t = dctxp.tile([P, 4, 512], F8, tag="dctx", name="dctx_t")
                        nc.gpsimd.dma_start(
                            t,
                            src[
                                4 * i * P : (4 * i + 4) * P, col_off : col_off + 512
                            ].rearrange("(a p) q -> p a q", p=P),
                        )
                        tiles.append(t)
                    return tiles

                def attn_block(b, qc, hl, aps, attp, extra_mm, sco_bufs=2):
                    """Attention for (b, head hl, q-chunk qc), k-tiles in
                    pairs; extra_mm(kp) emits a few independent matmuls per
                    pair to keep the PE busy while ACT/DVE run.  The scores
                    psum is split per 512-wide half so Exp(kp) releases each
                    half early and scores(kp+1) never stalls on ACT."""
                    qk = qk_sbs[b]
                    ctx_ps = aps.tile([P, 512], F32, tag="ctx", bufs=2)
                    sum_ps = aps.tile([P, 512], F32, tag="sum", bufs=1)
                    for kp in range(8):
                        kt0 = 2 * kp
                        exp_t = attp.tile([P, 1024], BF16, tag="exp")
                        for u in range(2):
                            s_ps = aps.tile(
                                [P, 512], F32, tag="sco", bufs=sco_bufs,
                            )
                            nc.tensor.matmul(
                                s_ps,
                                lhsT=qk[:, hl * 2 + 1, (kt0 + u) * P : (kt0 + u + 1) * P],
                                rhs=qk[:, hl * 2, qc * 512 : (qc + 1) * 512],
                                start=True,
                                stop=True,
                            )
                            nc.scalar.activation(
                                exp_t[:, u * 512 : (u + 1) * 512], s_ps, AF.Exp
                            )
                        prob_t = attp.tile([P, 1024], F8P, tag="prob")
                        nc.vector.tensor_mul(
                            prob_t.rearrange("p (u q) -> p u q", u=2),
                            exp_t.rearrange("p (u q) -> p u q", u=2),
                            mask_sb[:, kt0 : kt0 + 2, qc * 512 : (qc + 1) * 512],
                        )
                        pr = prob_t.rearrange("p (u q) -> p u q", u=2)
                        nc.tensor.matmul(
                            ctx_ps,
                            lhsT=v_sbs[b][:, hl, kt0 : kt0 + 2, :],
                            rhs=pr,
                            start=(kp == 0),
                            stop=(kp == 7),
                            perf_mode=DRM,
                        )
                        nc.tensor.matmul(
                            sum_ps,
                            lhsT=ow_sbs[b][:, hl, kt0 : kt0 + 2, :],
                            rhs=pr,
                            start=(kp == 0),
                            stop=(kp == 7),
                            perf_mode=DRM,
                        )
                        extra_mm(kp)
                    rec_t = attp.tile([P, 512], F32, tag="rec", bufs=3)
                    nc.vector.reciprocal_approx_fast(rec_t, sum_ps)
                    ctxn_t = attp.tile([P, 512], F8, tag="ctxn", bufs=5)
                    nc.vector.tensor_mul(ctxn_t, ctx_ps, rec_t)
                    dma_ctx(b, qc, hl, ctxn_t)

                # ---------- phase 1: QKV(b0), standalone ----------
                with (
                    tc.tile_pool(name="qk0", bufs=1) as qk0p,
                    tc.tile_pool(name="v0", bufs=1) as v0p,
                    tc.tile_pool(name="wq", bufs=1) as wqp,
                    tc.tile_pool(name="hid", bufs=2) as hidp,
                    tc.tile_pool(name="qps", bufs=3, space="PSUM") as qps,
                ):
                    qk_sbs[0] = qk0p.tile([P, 2 * HPC, S], BF16, name="qksb0")
                    v_sbs[0] = v0p.tile([P, HPC, 16, P], F8, name="vsb0")
                    wq_sb = wqp.tile([P, 16, OSH], F8)

                    def qkv_sc(b, sc, vT_sb):
                        """QKV for one 512-wide s-chunk: 6 o-tiles x 8 h-tile
                        PAIRS (DoubleRow); call emit(j) for j in range(48).
                        V^T o-tiles are PE-transposed to V [k, d] and scaled by
                        exp(alibi[k]); ones' tiles built alongside."""
                        if b == 0 and sc == 0:
                            # 8 contiguous pair-chunks: the first MM only
                            # needs the first 192KB chunk
                            for w in range(8):
                                nc.gpsimd.dma_start(
                                    wq_sb[:, 2 * w : 2 * (w + 1), :],
                                    wqkvT[:, 2 * w * OSH : 2 * (w + 1) * OSH].rearrange(
                                        "p (ht o) -> p ht o", ht=2
                                    ),
                                )
                        hid_t = hidp.tile([P, 8, 2, 512], F8, tag="hid")
                        i = b * 4 + sc
                        # two half-loads: MMs for h-pairs 0-3 start after 512KB
                        for w in range(2):
                            nc.sync.dma_start(
                                hid_t[:, 4 * w : 4 * (w + 1), :, :],
                                hidT[
                                    i * P : (i + 1) * P,
                                    w * 4096 : (w + 1) * 4096,
                                ].rearrange("p (hp a q) -> p hp a q", hp=4, a=2),
                            )
                        hid_ts = [hid_t[:, hp, :, :] for hp in range(8)]
                        state = {"ps": None}

                        def emit(j):
                            ot, hp = divmod(j, 8)
                            hl, t = divmod(ot, 3)
                            if hp == 0:
                                state["ps"] = qps.tile(
                                    [P, 512], F32, tag="qkvps", bufs=3,
                                    name=f"qps_{b}_{sc}_{ot}",
                                )
                            nc.tensor.matmul(
                                state["ps"],
                                lhsT=wq_sb[:, 2 * hp : 2 * hp + 2, ot * P : (ot + 1) * P],
                                rhs=hid_ts[hp],
                                start=(hp == 0),
                                stop=(hp == 7),
                                perf_mode=DRM,
                            )
                            if hp == 7:
                                # evacuate on DVE: keeps ScalarE exclusively on
                                # Exp; mult folds 1/WS (and inv_norm for q)
                                dst = (
                                    vT_sb[:, hl, sc * 512 : (sc + 1) * 512]
                                    if t == 2
                                    else qk_sbs[b][:, hl * 2 + t, sc * 512 : (sc + 1) * 512]
                                )
                                nc.vector.tensor_scalar(
                                    dst, state["ps"],
                                    (INV / WS) if t == 0 else (1.0 / WS),
                                    bq_sb[:, ot : ot + 1],
                                    ALU.mult, ALU.add,
                                )
                                if t == 2:
                                    # V^T chunk ready: PE-transpose its 4
                                    # k-tiles (psum slots borrowed from the
                                    # qkv pool) and scale rows by exp(alibi)
                                    for kk in range(4):
                                        kt = sc * 4 + kk
                                        acol = (b * HPC + hl) * 16 + kt
                                        vt_ps = qps.tile(
                                            [P, P], BF16, tag="qkvps", bufs=3,
                                            name=f"vt_{b}_{sc}_{hl}_{kk}",
                                        )
                                        nc.tensor.transpose(
                                            vt_ps,
                                            vT_sb[:, hl, kt * P : (kt + 1) * P],
                                            eye_sb,
                                        )
                                        nc.vector.tensor_scalar_mul(
                                            v_sbs[b][:, hl, kt, :],
                                            vt_ps,
                                            ale_sb[:, acol : acol + 1],
                                        )

                        return emit

                    vT0 = vtp.tile([P, HPC, S], BF16, tag="vT", name="vT0")
                    for sc in range(4):
                        emit = qkv_sc(0, sc, vT0)
                        for j in range(48):
                            emit(j)
                    # mask + denominator-weight loads on the idle gpsimd SWDGE
                    # queues, deferred so they don't steal head bandwidth from
                    # wq/hid (both are needed at phase-2 start)
                    for kt in range(16):
                        nc.gpsimd.dma_start(
                            mask_sb[:, kt, :], mask01T[kt * P : (kt + 1) * P, :]
                        )
                    for b in range(B):
                        nc.gpsimd.dma_start(
                            ow_sbs[b].rearrange("p hl kt q -> p (hl kt q)"),
                            owe[:, b * HPC * 16 * P : (b + 1) * HPC * 16 * P],
                        )

                    # ---------- phase 2: attention(b0) + QKV(b1) ----------
                    with (
                        tc.tile_pool(name="att", bufs=3) as attp,
                        tc.tile_pool(name="aps", bufs=1, space="PSUM") as aps,
                    ):
                        vT1 = vtp.tile([P, HPC, S], BF16, tag="vT", name="vT1")
                        pre_dctx = {}
                        # dense inputs load at phase-2 start: this window is
                        # clear of both the phase-1 hid burst and the first
                        # gather's fabric traffic
                        nc.gpsimd.dma_start(
                            wd_sb, wdT.rearrange("p (ht o) -> p ht o", ht=16)
                        )
                        nc.gpsimd.dma_start(
                            rs_sb, residT.rearrange("(ot p) s -> p ot s", p=P)
                        )
                        for qc in range(4):
                            for hl in range(HPC):
                                # 24 QKV(b1) matmuls woven into each block:
                                # 3 MMs per k-tile pair.
                                if hl == 0:
                                    emit = qkv_sc(1, qc, vT1)
                                base = 24 * hl

                                def extra(kp, emit=emit, base=base):
                                    for j in range(3):
                                        emit(base + kp * 3 + j)

                                attn_block(0, qc, hl, aps, attp, extra)
                            all_gather(0, qc)

                # ---------- phase 3: attention(b1) + dense(b0 + b1 early) --
                with (
                    tc.tile_pool(name="dps", bufs=2, space="PSUM") as dps,
                    tc.tile_pool(name="dout", bufs=3) as doutp,
                ):
                    # prefetch the first two dense chunks now: their gather
                    # landed during phase 2, so the scalar DMA queue never
                    # head-of-line blocks the Exp stream
                    pre_dctx[0] = dense_load(0)
                    pre_dctx[1] = dense_load(1)

                    def dense_sc(sc):
                        """One 512-wide output column chunk: 2 o-tiles x 8
                        h-tile pairs (DoubleRow), o-tile-major so only one
                        psum is live; emit(j) for j in range(16)."""
                        tiles = pre_dctx.pop(sc, None) or dense_load(sc)
                        state = {}

                        def emit(j):
                            ot, hp = divmod(j, 8)
                            if hp == 0:
                                state["ps"] = dps.tile(
                                    [P, 512], F32, tag="dps", bufs=2,
                                    name=f"dps_{sc}_{ot}",
                                )
                            nc.tensor.matmul(
                                state["ps"],
                                lhsT=wd_sb[:, 2 * hp : 2 * hp + 2, ot * P : (ot + 1) * P],
                                rhs=tiles[hp // 2][:, 2 * (hp % 2) : 2 * (hp % 2) + 2, :],
                                start=(hp == 0),
                                stop=(hp == 7),
                                perf_mode=DRM,
                            )
                            if hp == 7:
                                o_t = doutp.tile([P, 512], F32, tag="o")
                                nc.vector.scalar_tensor_tensor(
                                    o_t,
                                    state["ps"],
                                    1.0 / WS,
                                    rs_sb[:, ot, sc * 512 : (sc + 1) * 512],
                                    ALU.mult, ALU.add,
                                )
                                nc.sync.dma_start(
                                    outT[ot * P : (ot + 1) * P, sc * 512 : (sc + 1) * 512],
                                    o_t,
                                )

                        return emit

                    with (
                        tc.tile_pool(name="att1", bufs=3) as attp,
                        tc.tile_pool(name="aps1", bufs=1, space="PSUM") as aps,
                    ):
                        # blocks 0..7 = (qc, hl); dense chunks sc0..sc6
                        # woven into blocks 2..7 (sc5+sc6 both in block 7:
                        # a chunk's program position must precede the NEXT
                        # all_gather call, else the framework's conservative
                        # CC-semaphore wait makes its loads wait for ALL
                        # issued gathers — sc6 in phase 4 waited on the last
                        # gather despite its own landing 26us earlier).
                        DENSE_AT = {2: 0, 3: 1, 4: 2, 5: 3, 6: 4}
                        for qc in range(4):
                            for hl in range(HPC):
                                blk = qc * 2 + hl
                                if blk in DENSE_AT:
                                    emit = dense_sc(DENSE_AT[blk])

                                    def extra(kp, emit=emit):
                                        for j in range(2):
                                            emit(kp * 2 + j)
                                elif blk == 7:
                                    emit5 = dense_sc(5)
                                    emit6 = dense_sc(6)

                                    def extra(kp, e5=emit5, e6=emit6):
                                        for j in range(2):
                                            e5(kp * 2 + j)
                                            e6(kp * 2 + j)
                                else:
                                    def extra(kp):
                                        pass
                                attn_block(1, qc, hl, aps, attp, extra, sco_bufs=3)
                            # gather this q-chunk's ctx as soon as the second
                            # head finishes it
                            all_gather(1, qc)

                    # ---------- phase 4: dense tail (last b1 column) -------
                    for sc in range(7, 8):
                        emit = dense_sc(sc)
                        for j in range(16):
                            emit(j)

    nc.compile()
    return nc


def _prep_in_maps(hidden_states, residual, alibi, attention_mask, w_qkv, b_qkv, w_dense, b_dense):
    f32 = np.float32
    hs = np.asarray(hidden_states, f32).reshape(B * S, H)
    # [(b sc p), (hp a q)]: hidT_sw[(b,sc),p, hp,a,q] = hs.T[(2hp+a)*128+p,
    # b*S+sc*512+q] — each 512-wide s-chunk is contiguous for the DMA
    hsT = np.ascontiguousarray(hs.T).astype(NPE4)          # [H, B*S]
    hidT = (
        hsT.reshape(8, 2, P, B, 4, 512)                     # hp a p b sc q
        .transpose(3, 4, 2, 0, 1, 5)                        # b sc p hp a q
        .reshape(B * 4 * P, 8 * 2 * 512)
    )
    mask_keep = ~np.asarray(attention_mask).reshape(S, S)
    mask01T = np.ascontiguousarray(mask_keep.T).astype(NPE4)
    al = np.asarray(alibi, f32).reshape(B, NH, S)
    resid = np.asarray(residual, f32).reshape(B * S, H)
    wq = np.asarray(w_qkv, f32)
    bq = np.asarray(b_qkv, f32)
    wd = np.asarray(w_dense, f32)
    bd = np.asarray(b_dense, f32)
    inv = f32(INV)

    in_maps = []
    for r in range(NCORES):
        wshard = wq[r * OSH : (r + 1) * OSH] * WS
        bshard = bq[r * OSH : (r + 1) * OSH].copy()
        for hl in range(HPC):
            # q evacuation multiplies psum by inv/WS and adds bias*inv
            bshard[hl * 3 * HD : hl * 3 * HD + HD] *= inv
        alcols = []
        for b in range(B):
            for hl in range(HPC):
                alcols.append(np.exp(al[b, HPC * r + hl]).reshape(16, P).T)
        # denominator weights: exp(alibi[k]) broadcast across 128 cols,
        # laid out [P, (b hl kt col)] to match ow_sbs
        owe_np = np.concatenate(
            [np.broadcast_to(c[:, :, None], (P, 16, P)).reshape(P, -1) for c in alcols],
            axis=1,
        )
        # weights pre-swizzled to the SBUF tile layout [p, ht, o]
        wq_sw = (
            wshard.T.reshape(16, P, OSH).transpose(1, 0, 2).reshape(P, 16 * OSH)
        )
        wd_sw = (
            (wd[r * DSH : (r + 1) * DSH] * WS)
            .T.reshape(16, P, DSH).transpose(1, 0, 2).reshape(P, 16 * DSH)
        )
        in_maps.append(
            {
                "hidT": hidT,
                "wqkvT": np.ascontiguousarray(wq_sw).astype(NPE4),
                "bqkv": np.ascontiguousarray(bshard.reshape(6, P).T),
                "mask01T": mask01T,
                "alibi_e": np.ascontiguousarray(np.concatenate(alcols, axis=1)),
                "wdT": np.ascontiguousarray(wd_sw).astype(NPE4),
                "residT": np.ascontiguousarray(resid[:, r * DSH : (r + 1) * DSH].T)
                + bd[r * DSH : (r + 1) * DSH][:, None],
                "owe": np.ascontiguousarray(owe_np).astype(NPE4),
                "eye": np.eye(P, dtype=f32).astype(NPBF16),
            }
        )
    return in_maps


if os.environ.get("BASS_LDW_OPT"):
    _orig_run_command = bass_utils.run_command

    def _run_command_ldwopt(argv, **kwargs):
        argv = [
            "--enable-ldw-opt=true" if a == "--enable-ldw-opt=false" else a
            for a in argv
        ]
        return _orig_run_command(argv, **kwargs)

    bass_utils.run_command = _run_command_ldwopt


_NC_CACHE = {}


def run(inputs: dict, trace: bool = False):
    in_maps = _prep_in_maps(**inputs)
    if "nc" not in _NC_CACHE:
        _NC_CACHE["nc"] = build_nc()
    nc = _NC_CACHE["nc"]
    res = bass_utils.run_bass_kernel_spmd(
        nc, in_maps, core_ids=list(range(NCORES)), trace=trace
    )
    out = np.empty((B * S, H), np.float32)
    for r in range(NCORES):
        out[:, r * DSH : (r + 1) * DSH] = res.results[r]["outT"].T
    return out.reshape(B, S, H), res


def kernel(**inputs) -> np.ndarray:
    out, _ = run(inputs, trace=False)
    return out


# revision 48
# speedup vs baseline: 1.0898x; 1.0898x over previous
# Bloom parallel attention block on 8 trn2 NeuronCores, tensor-parallel over
# heads (2 heads per core).  Feature-major layouts throughout.
#
# v2: fp8 DoubleRow on every contraction>=256 matmul (QKV, ctx, denominator,
# dense) — two 128-deep k-tiles per PE instruction.  Scores stay bf16 (softmax
# is the accuracy-critical path).  Weights are prescaled x32 on host so their
# ~N(0, 1/2048) entries clear e4m3's 2^-6 min-normal; the 1/32 is folded into
# the PSUM evacuation (DVE tensor_scalar mult+add).  Probs are e5m2 (range:
# exp(scores) spans e4m3's +-240 but not e5m2's +-57344).  The ctx AllGather
# moves fp8 bytes (half of bf16), split into 8 per-(batch,q-chunk) ops so
# each triggers as soon as its two head-blocks finish and the serial CC
# stream stays spread out (the last gather is what exposes the tail).
#
# Scheduling notes (hard-won):
#  - DMAs that WAIT on a collective's semaphore must ride the gpsimd queue.
#    The static scheduler places them optimistically early in whatever engine
#    stream they're on; on the strict-FIFO scalar/sync queues they then
#    head-of-line block Exp / ctx-write instructions for the gather's full
#    latency (a 20-37us PE stall).  On gpsimd everything behind them (later
#    collective triggers) is already dependency-ordered after the gather.
#  - hid/wq/wd are host-pre-swizzled to per-partition-contiguous layouts:
#    scattered 512-768B DMA segments starved the phase-1 weave.
#  - wd/rs load at phase-2 start: phase 1 is HBM-tight, and mid-phase-2
#    collides with the first gather's fabric traffic.
#
# Per core r (heads 2r, 2r+1):
#   QKV matmul -> Q^T/K^T [d, s] (bf16, inv_norm applied at evacuation) and
#   V^T [d, s] per batch in SBUF.
#   V^T is transposed on the PE to V [s, d] and scaled by exp(alibi[k]) on
#   evacuation; the softmax-denominator matmul weights are exp(alibi[k])
#   broadcast columns ("ones'").  This folds alibi in MULTIPLICATIVELY:
#     exp(s + a) * mask = exp(s) * mask * exp(a)
#   so the ACT Exp needs no per-k-tile bias and can process two k-tiles per
#   instruction.
#   attention (per b, head hl, 512-wide q-chunk qc), scores transposed [k, q]:
#     scores^T = K^T_tile.T @ Q^T       per k-tile  (PE bf16, fp32 psum)
#     exp(scores) per k-tile            (ACT, bf16 out; per-512 psum halves
#                                        so scores(kp+1) never stalls on ACT)
#     * mask01^T                        (DVE, 0/1 mask -> e5m2 probs; tried
#                                        additive pre-exp mask: DVE lands in
#                                        the sco->exp critical path, -85us)
#     ctx^T += V'_pair.T @ probs        (PE fp8 DoubleRow, 8 k-pair steps)
#     sum   += ones'_pair.T @ probs     (PE fp8 DoubleRow, denominator)
#     ctx^T *= 1/sum -> e4m3 -> DMA to cc chunk
#   Pipelining: QKV(b1) matmuls are interleaved into attention(b0) k-loops,
#   dense matmuls into attention(b1) k-loops, so the PE never idles.  ctx
#   is AllGathered (fp8) in 8 per-(batch,q-chunk) ops; dense consumes each
#   as it lands (only the last chunk's dense remains after the attention).
#   dense: out^T[o_local, s] = wdT_pair.T @ ctx^T_full (DoubleRow), then
#     out = psum/32 + (residual^T + b_dense)  (DVE scalar_tensor_tensor)
import os
import sys

import numpy as np

if "/opt/trn_rl_repo" not in sys.path:
    sys.path.insert(0, "/opt/trn_rl_repo")

import ml_dtypes

import concourse.bass as bass
import concourse.mybir as mybir
import concourse.tile as tile
from concourse import bacc, bass_utils

B, S, H, NH = 2, 2048, 2048, 16
HD = H // NH            # 128
NCORES = 8
HPC = NH // NCORES      # heads per core = 2
OSH = 3 * H // NCORES   # qkv output rows per core = 768
DSH = H // NCORES       # dense output cols per core = 256
P = 128
F32 = mybir.dt.float32
BF16 = mybir.dt.bfloat16
F8 = mybir.dt.float8e4
F8P = mybir.dt.float8e5
DRM = mybir.MatmulPerfMode.DoubleRow
AF = mybir.ActivationFunctionType
ALU = mybir.AluOpType
NPBF16 = ml_dtypes.bfloat16
NPE4 = ml_dtypes.float8_e4m3
WS = 32.0               # host weight prescale (fp8 subnormal avoidance)
INV = 1.0 / np.sqrt(HD)


def build_nc():
    nc = bacc.Bacc(
        "TRN2",
        target_bir_lowering=False,
        debug=False,
        num_devices=NCORES,
    )

    # hidden states pre-swizzled on host to [(b sc p), (hp a q)] so each
    # 512-wide s-chunk is one fully-contiguous 1MB DMA
    hidT = nc.dram_tensor("hidT", [B * 4 * P, 8 * 2 * 512], F8, kind="ExternalInput").ap()
    # weights pre-swizzled on host to the SBUF tile layout [p, ht, o] so the
    # load is one fully-contiguous DMA per partition
    wqkvT = nc.dram_tensor("wqkvT", [P, 16 * OSH], F8, kind="ExternalInput").ap()
    bqkv = nc.dram_tensor("bqkv", [P, 6], F32, kind="ExternalInput").ap()
    mask01T = nc.dram_tensor("mask01T", [S, S], F8, kind="ExternalInput").ap()
    alibi_e = nc.dram_tensor("alibi_e", [P, 2 * HPC * 16], F32, kind="ExternalInput").ap()
    wdT = nc.dram_tensor("wdT", [P, 16 * DSH], F8, kind="ExternalInput").ap()
    residT = nc.dram_tensor("residT", [DSH, B * S], F32, kind="ExternalInput").ap()
    # host-prebuilt denominator weights exp(alibi[k]) broadcast to 128 cols
    owe = nc.dram_tensor("owe", [P, B * HPC * 16 * P], F8, kind="ExternalInput").ap()
    eye = nc.dram_tensor("eye", [P, P], BF16, kind="ExternalInput").ap()
    outT = nc.dram_tensor("outT", [DSH, B * S], F32, kind="ExternalOutput").ap()

    with tile.TileContext(nc) as tc:
        ccg = [list(range(NCORES))]
        with (
            tc.tile_pool(name="const", bufs=1) as constp,
            tc.tile_pool(name="dram", bufs=1, space="DRAM") as dramp,
        ):
            # consts ride the idle scalar queue so the gpsimd queue head is
            # free for the wq bulk loads (first-matmul critical path)
            bq_sb = constp.tile([P, 6], F32)
            nc.scalar.dma_start(bq_sb, bqkv)
            ale_sb = constp.tile([P, 2 * HPC * 16], F32)
            nc.scalar.dma_start(ale_sb, alibi_e)
            eye_sb = constp.tile([P, P], BF16)
            nc.scalar.dma_start(eye_sb, eye)

            # ctx gather chunks: one 512-col gather per (batch, q-chunk) —
            # small ops trigger early and keep the serial CC stream spread
            # out, so the phase-3 gather chain never queues behind a big op
            # and the tail drains quickly.
            cc_spec = [(4, S // 4), (4, S // 4)]
            cc_in = [
                [
                    dramp.tile([HPC * HD, w], F8, name=f"cc_in{b}{i}")
                    for i in range(n)
                ]
                for b, (n, w) in enumerate(cc_spec)
            ]
            cc_out = [
                [
                    dramp.tile([H, w], F8, addr_space="Shared", name=f"cc_out{b}{i}")
                    for i in range(n)
                ]
                for b, (n, w) in enumerate(cc_spec)
            ]

            def dma_ctx(b, qc, hl, ctxn_t):
                n, w = cc_spec[b]
                chunk, qq = divmod(qc, 4 // n)
                nc.sync.dma_start(
                    cc_in[b][chunk][hl * P : (hl + 1) * P, qq * 512 : (qq + 1) * 512],
                    ctxn_t,
                )

            def all_gather(b, chunk):
                nc.gpsimd.collective_compute(
                    "AllGather", mybir.AluOpType.bypass, replica_groups=ccg,
                    ins=[cc_in[b][chunk].opt()], outs=[cc_out[b][chunk].opt()],
                )

            with (
                tc.tile_pool(name="mask", bufs=1) as maskp,
                tc.tile_pool(name="qk1", bufs=1) as qk1p,
                tc.tile_pool(name="vt", bufs=1) as vtp,
                tc.tile_pool(name="v1", bufs=1) as v1p,
                tc.tile_pool(name="ow0", bufs=1) as ow0p,
                tc.tile_pool(name="ow1", bufs=1) as ow1p,
                tc.tile_pool(name="dw", bufs=1) as dwp,
                tc.tile_pool(name="dctx", bufs=12) as dctxp,
            ):
                mask_sb = maskp.tile([P, 16, S], F8)
                qk_sbs = [None, qk1p.tile([P, 2 * HPC, S], BF16, name="qksb1")]
                v_sbs = [None, v1p.tile([P, HPC, 16, P], F8, name="vsb1")]
                ow_sbs = [
                    ow0p.tile([P, HPC, 16, P], F8, name="owsb0"),
                    ow1p.tile([P, HPC, 16, P], F8, name="owsb1"),
                ]
                wd_sb = dwp.tile([P, 16, DSH], F8)
                rs_sb = dwp.tile([P, 2, B * S], F32)

                def dense_src(sc):
                    """cc_out chunk + column offset for output chunk sc."""
                    if sc < 4:
                        return cc_out[0][sc], 0
                    return cc_out[1][sc - 4], 0

                def dense_load(sc):
                    """Stage the gathered ctx for output chunk sc into SBUF:
                    4 tiles of 4 h-tiles each.  MUST ride the gpsimd queue:
                    these DMAs wait on a collective's completion semaphore,
                    and the static scheduler places them optimistically early
                    in whatever engine stream they're on — on scalar/sync the
                    strict-FIFO queue then head-of-line blocks Exp/ctx-write
                    instructions for the gather's full latency.  On gpsimd
                    everything behind them (later collective triggers) is
                    already dependency-ordered after the gather they wait on."""
                    src, col_off = dense_src(sc)
                    tiles = []
                    for i in range(4):
                        t = dctxp.tile([P, 4, 512], F8, tag="dctx", name="dctx_t")
                        nc.gpsimd.dma_start(
                            t,
                            src[
                                4 * i * P : (4 * i + 4) * P, col_off : col_off + 512
                            ].rearrange("(a p) q -> p a q", p=P),
                        )
                        tiles.append(t)
                    return tiles

                def attn_block(b, qc, hl, aps, attp, extra_mm, sco_bufs=2):
                    """Attention for (b, head hl, q-chunk qc), k-tiles in
                    pairs; extra_mm(kp) emits a few independent matmuls per
                    pair to keep the PE busy while ACT/DVE run.  The scores
                    psum is split per 512-wide half so Exp(kp) releases each
                    half early and scores(kp+1) never stalls on ACT."""
                    qk = qk_sbs[b]
                    ctx_ps = aps.tile([P, 512], F32, tag="ctx", bufs=2)
                    sum_ps = aps.tile([P, 512], F32, tag="sum", bufs=1)
                    for kp in range(8):
                        kt0 = 2 * kp
                        exp_t = attp.tile([P, 1024], BF16, tag="exp")
                        for u in range(2):
                            s_ps = aps.tile(
                                [P, 512], F32, tag="sco", bufs=sco_bufs,
                            )
                            nc.tensor.matmul(
                                s_ps,
                                lhsT=qk[:, hl * 2 + 1, (kt0 + u) * P : (kt0 + u + 1) * P],
                                rhs=qk[:, hl * 2, qc * 512 : (qc + 1) * 512],
                                start=True,
                                stop=True,
                            )
                            nc.scalar.activation(
                                exp_t[:, u * 512 : (u + 1) * 512], s_ps, AF.Exp
                            )
                        prob_t = attp.tile([P, 1024], F8P, tag="prob")
                        nc.vector.tensor_mul(
                            prob_t.rearrange("p (u q) -> p u q", u=2),
                            exp_t.rearrange("p (u q) -> p u q", u=2),
                            mask_sb[:, kt0 : kt0 + 2, qc * 512 : (qc + 1) * 512],
                        )
                        pr = prob_t.rearrange("p (u q) -> p u q", u=2)
                        nc.tensor.matmul(
                            ctx_ps,
                            lhsT=v_sbs[b][:, hl, kt0 : kt0 + 2, :],
                            rhs=pr,
                            start=(kp == 0),
                            stop=(kp == 7),
                            perf_mode=DRM,
                        )
                        nc.tensor.matmul(
                            sum_ps,
                            lhsT=ow_sbs[b][:, hl, kt0 : kt0 + 2, :],
                            rhs=pr,
                            start=(kp == 0),
                            stop=(kp == 7),
                            perf_mode=DRM,
                        )
                        extra_mm(kp)
                    rec_t = attp.tile([P, 512], F32, tag="rec", bufs=3)
                    nc.vector.reciprocal_approx_fast(rec_t, sum_ps)
                    ctxn_t = attp.tile([P, 512], F8, tag="ctxn", bufs=5)
                    nc.vector.tensor_mul(ctxn_t, ctx_ps, rec_t)
                    dma_ctx(b, qc, hl, ctxn_t)

                # ---------- phase 1: QKV(b0), standalone ----------
                with (
                    tc.tile_pool(name="qk0", bufs=1) as qk0p,
                    tc.tile_pool(name="v0", bufs=1) as v0p,
                    tc.tile_pool(name="wq", bufs=1) as wqp,
                    tc.tile_pool(name="hid", bufs=2) as hidp,
                    tc.tile_pool(name="qps", bufs=3, space="PSUM") as qps,
                ):
                    qk_sbs[0] = qk0p.tile([P, 2 * HPC, S], BF16, name="qksb0")
                    v_sbs[0] = v0p.tile([P, HPC, 16, P], F8, name="vsb0")
                    wq_sb = wqp.tile([P, 16, OSH], F8)

                    def qkv_sc(b, sc, vT_sb):
                        """QKV for one 512-wide s-chunk: 6 o-tiles x 8 h-tile
                        PAIRS (DoubleRow); call emit(j) for j in range(48).
                        V^T o-tiles are PE-transposed to V [k, d] and scaled by
                        exp(alibi[k]); ones' tiles built alongside."""
                        if b == 0 and sc == 0:
                            # 8 contiguous pair-chunks: the first MM only
                            # needs the first 192KB chunk
                            for w in range(8):
                                nc.gpsimd.dma_start(
                                    wq_sb[:, 2 * w : 2 * (w + 1), :],
                                    wqkvT[:, 2 * w * OSH : 2 * (w + 1) * OSH].rearrange(
                                        "p (ht o) -> p ht o", ht=2
                                    ),
                                )
                        hid_t = hidp.tile([P, 8, 2, 512], F8, tag="hid")
                        i = b * 4 + sc
                        nc.sync.dma_start(
                            hid_t,
                            hidT[i * P : (i + 1) * P, :].rearrange(
                                "p (hp a q) -> p hp a q", hp=8, a=2
                            ),
                        )
                        hid_ts = [hid_t[:, hp, :, :] for hp in range(8)]
                        state = {"ps": None}

                        def emit(j):
                            ot, hp = divmod(j, 8)
                            hl, t = divmod(ot, 3)
                            if hp == 0:
                                state["ps"] = qps.tile(
                                    [P, 512], F32, tag="qkvps", bufs=3,
                                    name=f"qps_{b}_{sc}_{ot}",
                                )
                            nc.tensor.matmul(
                                state["ps"],
                                lhsT=wq_sb[:, 2 * hp : 2 * hp + 2, ot * P : (ot + 1) * P],
                                rhs=hid_ts[hp],
                                start=(hp == 0),
                                stop=(hp == 7),
                                perf_mode=DRM,
                            )
                            if hp == 7:
                                # evacuate on DVE: keeps ScalarE exclusively on
                                # Exp; mult folds 1/WS (and inv_norm for q)
                                dst = (
                                    vT_sb[:, hl, sc * 512 : (sc + 1) * 512]
                                    if t == 2
                                    else qk_sbs[b][:, hl * 2 + t, sc * 512 : (sc + 1) * 512]
                                )
                                nc.vector.tensor_scalar(
                                    dst, state["ps"],
                                    (INV / WS) if t == 0 else (1.0 / WS),
                                    bq_sb[:, ot : ot + 1],
                                    ALU.mult, ALU.add,
                                )
                                if t == 2:
                                    # V^T chunk ready: PE-transpose its 4
                                    # k-tiles (psum slots borrowed from the
                                    # qkv pool) and scale rows by exp(alibi)
                                    for kk in range(4):
                                        kt = sc * 4 + kk
                                        acol = (b * HPC + hl) * 16 + kt
                                        vt_ps = qps.tile(
                                            [P, P], BF16, tag="qkvps", bufs=3,
                                            name=f"vt_{b}_{sc}_{hl}_{kk}",
                                        )
                                        nc.tensor.transpose(
                                            vt_ps,
                                            vT_sb[:, hl, kt * P : (kt + 1) * P],
                                            eye_sb,
                                        )
                                        nc.vector.tensor_scalar_mul(
                                            v_sbs[b][:, hl, kt, :],
                                            vt_ps,
                                            ale_sb[:, acol : acol + 1],
                                        )

                        return emit

                    vT0 = vtp.tile([P, HPC, S], BF16, tag="vT", name="vT0")
                    for sc in range(4):
                        emit = qkv_sc(0, sc, vT0)
                        for j in range(48):
                            emit(j)
                    # mask + denominator-weight loads on the idle gpsimd SWDGE
                    # queues, deferred so they don't steal head bandwidth from
                    # wq/hid (both are needed at phase-2 start)
                    for kt in range(16):
                        nc.gpsimd.dma_start(
                            mask_sb[:, kt, :], mask01T[kt * P : (kt + 1) * P, :]
                        )
                    for b in range(B):
                        nc.gpsimd.dma_start(
                            ow_sbs[b].rearrange("p hl kt q -> p (hl kt q)"),
                            owe[:, b * HPC * 16 * P : (b + 1) * HPC * 16 * P],
                        )

                    # ---------- phase 2: attention(b0) + QKV(b1) ----------
                    with (
                        tc.tile_pool(name="att", bufs=3) as attp,
                        tc.tile_pool(name="aps", bufs=1, space="PSUM") as aps,
                    ):
                        vT1 = vtp.tile([P, HPC, S], BF16, tag="vT", name="vT1")
                        pre_dctx = {}
                        # dense inputs load at phase-2 start: this window is
                        # clear of both the phase-1 hid burst and the first
                        # gather's fabric traffic
                        nc.gpsimd.dma_start(
                            wd_sb, wdT.rearrange("p (ht o) -> p ht o", ht=16)
                        )
                        nc.gpsimd.dma_start(
                            rs_sb, residT.rearrange("(ot p) s -> p ot s", p=P)
                        )
                        for qc in range(4):
                            for hl in range(HPC):
                                # 24 QKV(b1) matmuls woven into each block:
                                # 3 MMs per k-tile pair.
                                if hl == 0:
                                    emit = qkv_sc(1, qc, vT1)
                                base = 24 * hl

                                def extra(kp, emit=emit, base=base):
                                    for j in range(3):
                                        emit(base + kp * 3 + j)

                                attn_block(0, qc, hl, aps, attp, extra)
                            all_gather(0, qc)

                # ---------- phase 3: attention(b1) + dense(b0 + b1 early) --
                with (
                    tc.tile_pool(name="dps", bufs=2, space="PSUM") as dps,
                    tc.tile_pool(name="dout", bufs=3) as doutp,
                ):
                    # prefetch the first two dense chunks now: their gather
                    # landed during phase 2, so the scalar DMA queue never
                    # head-of-line blocks the Exp stream
                    pre_dctx[0] = dense_load(0)
                    pre_dctx[1] = dense_load(1)

                    def dense_sc(sc):
                        """One 512-wide output column chunk: 2 o-tiles x 8
                        h-tile pairs (DoubleRow), o-tile-major so only one
                        psum is live; emit(j) for j in range(16)."""
                        tiles = pre_dctx.pop(sc, None) or dense_load(sc)
                        state = {}

                        def emit(j):
                            ot, hp = divmod(j, 8)
                            if hp == 0:
                                state["ps"] = dps.tile(
                                    [P, 512], F32, tag="dps", bufs=2,
                                    name=f"dps_{sc}_{ot}",
                                )
                            nc.tensor.matmul(
                                state["ps"],
                                lhsT=wd_sb[:, 2 * hp : 2 * hp + 2, ot * P : (ot + 1) * P],
                                rhs=tiles[hp // 2][:, 2 * (hp % 2) : 2 * (hp % 2) + 2, :],
                                start=(hp == 0),
                                stop=(hp == 7),
                                perf_mode=DRM,
                            )
                            if hp == 7:
                                o_t = doutp.tile([P, 512], F32, tag="o")
                                nc.vector.scalar_tensor_tensor(
                                    o_t,
                                    state["ps"],
                                    1.0 / WS,
                                    rs_sb[:, ot, sc * 512 : (sc + 1) * 512],
                                    ALU.mult, ALU.add,
                                )
                                nc.sync.dma_start(
                                    outT[ot * P : (ot + 1) * P, sc * 512 : (sc + 1) * 512],
                                    o_t,
                                )

                        return emit

                    with (
                        tc.tile_pool(name="att1", bufs=3) as attp,
                        tc.tile_pool(name="aps1", bufs=1, space="PSUM") as aps,
                    ):
                        # blocks 0..7 = (qc, hl); dense chunks sc0..sc6
                        # woven into blocks 2..7 (sc5+sc6 both in block 7:
                        # a chunk's program position must precede the NEXT
                        # all_gather call, else the framework's conservative
                        # CC-semaphore wait makes its loads wait for ALL
                        # issued gathers — sc6 in phase 4 waited on the last
                        # gather despite its own landing 26us earlier).
                        DENSE_AT = {2: 0, 3: 1, 4: 2, 5: 3, 6: 4}
                        for qc in range(4):
                            for hl in range(HPC):
                                blk = qc * 2 + hl
                                if blk in DENSE_AT:
                                    emit = dense_sc(DENSE_AT[blk])

                                    def extra(kp, emit=emit):
                                        for j in range(2):
                                            emit(kp * 2 + j)
                                elif blk == 7:
                                    emit5 = dense_sc(5)
                                    emit6 = dense_sc(6)

                                    def extra(kp, e5=emit5, e6=emit6):
                                        for j in range(2):
                                            e5(kp * 2 + j)
                                            e6(kp * 2 + j)
                                else:
                                    def extra(kp):
                                        pass
                                attn_block(1, qc, hl, aps, attp, extra, sco_bufs=3)
                            # gather this q-chunk's ctx as soon as the second
                            # head finishes it
                            all_gather(1, qc)

                    # ---------- phase 4: dense tail (last b1 column) -------
                    for sc in range(7, 8):
                        emit = dense_sc(sc)
                        for j in range(16):
                            emit(j)

    nc.compile()
    return nc


def _prep_in_maps(hidden_states, residual, alibi, attention_mask, w_qkv, b_qkv, w_dense, b_dense):
    f32 = np.float32
    hs = np.asarray(hidden_states, f32).reshape(B * S, H)
    # [(b sc p), (hp a q)]: hidT_sw[(b,sc),p, hp,a,q] = hs.T[(2hp+a)*128+p,
    # b*S+sc*512+q] — each 512-wide s-chunk is contiguous for the DMA
    hsT = np.ascontiguousarray(hs.T).astype(NPE4)          # [H, B*S]
    hidT = (
        hsT.reshape(8, 2, P, B, 4, 512)                     # hp a p b sc q
        .transpose(3, 4, 2, 0, 1, 5)                        # b sc p hp a q
        .reshape(B * 4 * P, 8 * 2 * 512)
    )
    mask_keep = ~np.asarray(attention_mask).reshape(S, S)
    mask01T = np.ascontiguousarray(mask_keep.T).astype(NPE4)
    al = np.asarray(alibi, f32).reshape(B, NH, S)
    resid = np.asarray(residual, f32).reshape(B * S, H)
    wq = np.asarray(w_qkv, f32)
    bq = np.asarray(b_qkv, f32)
    wd = np.asarray(w_dense, f32)
    bd = np.asarray(b_dense, f32)
    inv = f32(INV)

    in_maps = []
    for r in range(NCORES):
        wshard = wq[r * OSH : (r + 1) * OSH] * WS
        bshard = bq[r * OSH : (r + 1) * OSH].copy()
        for hl in range(HPC):
            # q evacuation multiplies psum by inv/WS and adds bias*inv
            bshard[hl * 3 * HD : hl * 3 * HD + HD] *= inv
        alcols = []
        for b in range(B):
            for hl in range(HPC):
                alcols.append(np.exp(al[b, HPC * r + hl]).reshape(16, P).T)
        # denominator weights: exp(alibi[k]) broadcast across 128 cols,
        # laid out [P, (b hl kt col)] to match ow_sbs
        owe_np = np.concatenate(
            [np.broadcast_to(c[:, :, None], (P, 16, P)).reshape(P, -1) for c in alcols],
            axis=1,
        )
        # weights pre-swizzled to the SBUF tile layout [p, ht, o]
        wq_sw = (
            wshard.T.reshape(16, P, OSH).transpose(1, 0, 2).reshape(P, 16 * OSH)
        )
        wd_sw = (
            (wd[r * DSH : (r + 1) * DSH] * WS)
            .T.reshape(16, P, DSH).transpose(1, 0, 2).reshape(P, 16 * DSH)
        )
        in_maps.append(
            {
                "hidT": hidT,
                "wqkvT": np.ascontiguousarray(wq_sw).astype(NPE4),
                "bqkv": np.ascontiguousarray(bshard.reshape(6, P).T),
                "mask01T": mask01T,
                "alibi_e": np.ascontiguousarray(np.concatenate(alcols, axis=1)),
                "wdT": np.ascontiguousarray(wd_sw).astype(NPE4),
                "residT": np.ascontiguousarray(resid[:, r * DSH : (r + 1) * DSH].T)
                + bd[r * DSH : (r + 1) * DSH][:, None],
                "owe": np.ascontiguousarray(owe_np).astype(NPE4),
                "eye": np.eye(P, dtype=f32).astype(NPBF16),
            }
        )
    return in_maps


if os.environ.get("BASS_LDW_OPT"):
    _orig_run_command = bass_utils.run_command

    def _run_command_ldwopt(argv, **kwargs):
        argv = [
            "--enable-ldw-opt=true" if a == "--enable-ldw-opt=false" else a
            for a in argv
        ]
        return _orig_run_command(argv, **kwargs)

    bass_utils.run_command = _run_command_ldwopt


_NC_CACHE = {}


def run(inputs: dict, trace: bool = False):
    in_maps = _prep_in_maps(**inputs)
    if "nc" not in _NC_CACHE:
        _NC_CACHE["nc"] = build_nc()
    nc = _NC_CACHE["nc"]
    res = bass_utils.run_bass_kernel_spmd(
        nc, in_maps, core_ids=list(range(NCORES)), trace=trace
    )
    out = np.empty((B * S, H), np.float32)
    for r in range(NCORES):
        out[:, r * DSH : (r + 1) * DSH] = res.results[r]["outT"].T
    return out.reshape(B, S, H), res


def kernel(**inputs) -> np.ndarray:
    out, _ = run(inputs, trace=False)
    return out
